# revision 6
# baseline (speedup 1.0000x reference)
"""Trainium2 Bass kernel for the LogRatio loss (nn_LogRatio_14104672600201).

Math: the reference loss factorizes (see the derivation in the epilogue
below). Every masked reduction over logsim[j, l] = log((X @ X.T)[j, l]) has a
mask depending on j only through targets[j] in [0, 64), so each row-reduction
becomes a GEMM against a label-derived matrix Q[l, g] followed by a per-row
one-hot select at g = targets[j].

Layout (g-major): per core (512-row j-shard, rotated so its own rows sit at
columns 0:512 of the l axis):

  for each of 32 l-tiles:
    sim   = xT_tile.T @ xT[:, 0:512]            # [128 l, 512 j]  PSUM
    y     = Ln(KSC * sim)                       # shifted log, bf16
    y2    = y * y                               # DVE bf16 2x
    X1a  += Qa.T @ y      (Qa = [P | W0])       # [128 g, 512 j]  accum
    X2a  += Qa.T @ y2                           # [128 g, 512 j]  accum
    X1b  += Qb.T @ y      (Qb = [W1])           # [ 64 g, 512 j]  accum

Q is STATIONARY and the full j-shard is the MOVING operand (N=512), so the
PE runs ~134 big matmuls instead of 320 small ones, and LDWEIGHTS drops
3x. All matmul operands are bf16 (the shift trick keeps y in [-0.3, 0.35],
so bf16's 8-bit mantissa costs only ~1e-4 absolute per element).

Selection: the 5 per-j values (yP, yW0, yW1, y2P, y2W0) are extracted
without transposes: multiply the [g, j] accumulators elementwise by the
one-hot mask M[g, j] = (g % 64 == t_j), then contract over partitions with a
tiny 2-column indicator matmul -> [2, 512] rows. Those 6 rows DMA out and
the final scalar loss is reconstructed on host in float64.
"""

import numpy as np
import ml_dtypes

N, D, KK, C = 4096, 128, 4, 64
NCORES = 8
JSH = N // NCORES          # 512 j rows per core
LT = N // 128              # 32 l-tiles
GW = 192                   # Q width: [P(64) | W0(64) | W1(64)]
EPS = 1e-6
OMEGA = 0.1
KSC = float(np.float32(np.exp(-3.5)))        # Ln input scale (exactly f32)
SHIFT = float(-np.log(np.float64(KSC)))      # effective shift s = -ln(KSC)

_CACHE = {}


def _build_nc():
    import concourse.bass as bass
    import concourse.bacc as bacc
    import concourse.mybir as mybir
    import concourse.tile as tile
    from contextlib import ExitStack

    f32 = mybir.dt.float32
    f32r = mybir.dt.float32r
    bf16 = mybir.dt.bfloat16
    Ln = mybir.ActivationFunctionType.Ln

    nc = bacc.Bacc("TRN2", target_bir_lowering=False, debug=False)
    xt = nc.dram_tensor("xt", [D, N], bf16, kind="ExternalInput")
    # q[p, lt*GW + g] = Q[lt*128 + p, g]
    q = nc.dram_tensor("q", [128, LT * GW], bf16, kind="ExternalInput")
    # mask[g, j] = (g % 64 == t_j), stacked twice along g
    msk = nc.dram_tensor("msk", [128, JSH], bf16, kind="ExternalInput")
    # indicator: ind[g, 0] = (g < 64), ind[g, 1] = (g >= 64)
    ind = nc.dram_tensor("ind", [128, 2], bf16, kind="ExternalInput")
    # out rows: [yP, yW0 | yW1, 0 | y2P, y2W0]
    lout = nc.dram_tensor("lout", [2, 3 * JSH], f32, kind="ExternalOutput")

    NP = LT // 2  # sim tiles processed in pairs (two PSUM banks per Ln/sq)

    with tile.TileContext(nc) as tc, ExitStack() as ctx:
        cpool = ctx.enter_context(tc.tile_pool(name="const", bufs=1))
        work = ctx.enter_context(tc.tile_pool(name="work", bufs=3))
        mpool = ctx.enter_context(tc.tile_pool(name="mpool", bufs=1))
        psim = ctx.enter_context(tc.tile_pool(name="psim", bufs=2, space="PSUM"))
        px = ctx.enter_context(tc.tile_pool(name="px", bufs=1, space="PSUM"))
        psel = ctx.enter_context(tc.tile_pool(name="psel", bufs=1, space="PSUM"))

        # ---- DMAs: first wave feeds the first sim matmuls; issue from three
        # different engine queues (each DMA_DIRECT2D costs ~0.6us serial on
        # its queue). msk lands first and doubles as PE warm-up data.
        msk_sb = cpool.tile([128, JSH], bf16, tag="msk")
        nc.sync.dma_start(msk_sb[:], msk[:])
        xt_sb = cpool.tile([D, N], bf16, tag="xt")
        nc.sync.dma_start(xt_sb[:, 0:512], xt[:, 0:512])
        q_sb = cpool.tile([128, LT * GW], bf16, tag="q")
        nc.gpsimd.dma_start(q_sb[:, 0:1536], q[:, 0:1536])
        nc.sync.dma_start(xt_sb[:, 512:2048], xt[:, 512:2048])
        nc.gpsimd.dma_start(q_sb[:, 1536:3840], q[:, 1536:3840])
        nc.sync.dma_start(xt_sb[:, 2048:4096], xt[:, 2048:4096])
        nc.gpsimd.dma_start(q_sb[:, 3840:6144], q[:, 3840:6144])
        ind_sb = cpool.tile([128, 2], bf16, tag="ind")
        nc.gpsimd.dma_start(ind_sb[:], ind[:])

        # ---- PE warm-up: back-to-back dummy matmuls on the mask tile keep
        # the PE busy from ~1.5us so the HAM clock-gate opens (1.2 -> 2.4
        # GHz) before the real stream begins. Output is never read.
        warm = psel.tile([128, JSH], f32, tag="sel", name="warm")
        for _ in range(6):
            nc.tensor.matmul(
                warm[:], msk_sb[:, 0:128], msk_sb[:], start=True, stop=True
            )

        # accumulators (one PSUM bank each, held across the whole lt loop)
        x1a = px.tile([128, JSH], f32, tag="x1a")
        x2a = px.tile([128, JSH], f32, tag="x2a")
        x1b = px.tile([64, JSH], f32, tag="x1b")

        mov = xt_sb[:, 0:JSH]
        ls_t = [None] * NP
        ls2_t = [None] * NP

        def sim_stage(p):
            simp = psim.tile([128, 2 * JSH], f32, tag="simp", name=f"simp{p}")
            for h in range(2):
                nc.tensor.matmul(
                    simp[:, bass.ts(h, JSH)],
                    xt_sb[:, bass.ts(2 * p + h, 128)],
                    mov,
                    start=True,
                    stop=True,
                )
            ls = work.tile([128, 2 * JSH], bf16, tag="ls", name=f"ls{p}")
            nc.scalar.activation(ls[:], simp[:], Ln, scale=KSC)
            ls2 = work.tile([128, 2 * JSH], bf16, tag="ls2", name=f"ls2{p}")
            nc.vector.tensor_mul(ls2[:], ls[:], ls[:])
            ls_t[p] = ls
            ls2_t[p] = ls2

        sim_stage(0)
        sim_stage(1)
        for p in range(NP):
            if p + 2 < NP:
                sim_stage(p + 2)
            for h in range(2):
                lt = 2 * p + h
                qa = q_sb[:, lt * GW : lt * GW + 128]
                qb = q_sb[:, lt * GW + 128 : lt * GW + GW]
                y = ls_t[p][:, bass.ts(h, JSH)]
                y2 = ls2_t[p][:, bass.ts(h, JSH)]
                st = lt == 0
                sp = lt == LT - 1
                nc.tensor.matmul(x1a[:], qa, y, start=st, stop=sp)
                nc.tensor.matmul(x2a[:], qa, y2, start=st, stop=sp)
                nc.tensor.matmul(x1b[:], qb, y, start=st, stop=sp)

        # ---- selection: mask-mul then 2-column collapse matmul ----
        sel_sb = mpool.tile([2, 3 * JSH], f32, tag="selsb")
        for i, (src, prange) in enumerate(((x1a, 128), (x1b, 64), (x2a, 128))):
            m = mpool.tile([prange, JSH], bf16, tag=f"m{i}", name=f"m{i}")
            nc.vector.tensor_mul(m[:], src[0:prange, :], msk_sb[0:prange, :])
            sel = psel.tile([2, JSH], f32, tag="sel", name=f"sel{i}")
            nc.tensor.matmul(
                sel[:], ind_sb[0:prange, :], m[:], start=True, stop=True
            )
            nc.vector.tensor_copy(sel_sb[:, bass.ts(i, JSH)], sel[:])
        nc.sync.dma_start(lout[:], sel_sb[:])
    nc.compile()
    return nc


def _host_prep(inputs, labels):
    x = np.asarray(inputs, dtype=np.float32)
    lab = np.asarray(labels)
    t = lab[:, 0].astype(np.int64)
    bf = ml_dtypes.bfloat16

    m = np.arange(KK)
    om = np.float64(OMEGA)
    lp = np.log(np.float64(OMEGA + EPS)) - np.log(om ** (KK - m + 1) + np.float64(EPS))

    gr = np.arange(C)
    eq = lab[None, :, :] == gr[:, None, None]          # [C, N, KK]
    nm = np.stack(
        [
            ~eq[:, :, 3],
            eq[:, :, 3] & ~eq[:, :, 2],
            eq[:, :, 2] & ~eq[:, :, 1],
            eq[:, :, 1] & ~eq[:, :, 0],
        ]
    ).astype(np.float64)                                # [KK, C, N]
    w0 = nm.sum(0)                                      # [C, N]
    w1 = np.einsum("m,mcl->cl", lp, nm)
    w2 = np.einsum("m,mcl->cl", lp * lp, nm)
    ph = (t[:, None] == gr[None, :]).astype(np.float64)  # [N, C] one-hot t_l

    qm = np.zeros((N, GW), dtype=np.float32)
    qm[:, 0:C] = ph
    qm[:, C : 2 * C] = w0.T
    qm[:, 2 * C : 3 * C] = w1.T

    ind = np.zeros((128, 2), dtype=np.float32)
    ind[0:64, 0] = 1.0
    ind[64:128, 1] = 1.0

    xt = np.ascontiguousarray(x.T)                       # [D, N]
    in_maps = []
    for cid in range(NCORES):
        sl = slice(cid * JSH, (cid + 1) * JSH)
        # rotate the l axis so this core's own j-shard sits at columns
        # 0:JSH; the l reduction (over all 4096) is rotation-invariant as
        # long as q's rows rotate identically.
        xtc = np.roll(xt, -cid * JSH, axis=1)
        qc = np.roll(qm, -cid * JSH, axis=0)             # [N, GW]
        # q_sb[p, lt*GW + g] = Q[lt*128 + p, g]
        qsb = np.ascontiguousarray(
            qc.reshape(LT, 128, GW).transpose(1, 0, 2).reshape(128, LT * GW)
        )
        oh = (gr[:, None] == t[sl][None, :]).astype(np.float32)  # [64, 512]
        mk = np.concatenate([oh, oh], axis=0)            # [128, 512]
        in_maps.append(
            {
                "xt": xtc.astype(bf),
                "q": qsb.astype(bf),
                "msk": mk.astype(bf),
                "ind": ind.astype(bf),
            }
        )

    tabs = {
        "t": t, "cnt": ph.sum(0), "h0": w0.sum(1), "h1": w1.sum(1),
        "h2": w2.sum(1), "x": x,
    }
    return in_maps, tabs


def _host_loss(res_list, tabs):
    t, cnt, h0, h1, h2 = tabs["t"], tabs["cnt"], tabs["h0"], tabs["h1"], tabs["h2"]
    x64 = tabs["x"].astype(np.float64)
    s = np.float64(SHIFT)
    loss = np.float64(0.0)
    for cid, r in enumerate(res_list):
        sl = slice(cid * JSH, (cid + 1) * JSH)
        lo = r["lout"].astype(np.float64)                # [2, 3*JSH]
        yP, yW0 = lo[0, 0:JSH], lo[1, 0:JSH]
        yW1 = lo[0, JSH : 2 * JSH]
        y2P, y2W0 = lo[0, 2 * JSH :], lo[1, 2 * JSH :]
        tj = t[sl]
        cj, h0j, h1j, h2j = cnt[tj], h0[tj], h1[tj], h2[tj]
        diag = np.log(np.einsum("jd,jd->j", x64[sl], x64[sl]) + EPS)
        S1 = yP + s * cj - diag
        S2 = y2P + 2 * s * yP + s * s * cj - diag * diag
        A1 = yW0 + s * h0j + 0.1 * h1j
        A2 = (y2W0 + 2 * s * yW0 + s * s * h0j) + 0.2 * (yW1 + s * h1j) + 0.01 * h2j
        loss += np.sum(S2 * h0j - 2.0 * S1 * A1 + (cj - 1.0) * A2)
    return np.array(loss, dtype=np.float32)


def _run(inputs, labels, trace=False, tmpdir=None):
    from concourse.bass_utils import run_bass_kernel_spmd

    if "nc" not in _CACHE:
        _CACHE["nc"] = _build_nc()
    in_maps, tabs = _host_prep(inputs, labels)
    res = run_bass_kernel_spmd(
        _CACHE["nc"], in_maps, core_ids=list(range(NCORES)),
        trace=trace, tmpdir=tmpdir,
    )
    return _host_loss(res.results, tabs), res


def kernel(inputs, labels):
    out, _ = _run(inputs, labels, trace=False)
    return out


# revision 7
# speedup vs baseline: 1.0267x; 1.0267x over previous
"""Trainium2 Bass kernel for the LogRatio loss (nn_LogRatio_14104672600201).

Math: the reference loss factorizes (see the derivation in the epilogue
below). Every masked reduction over logsim[j, l] = log((X @ X.T)[j, l]) has a
mask depending on j only through targets[j] in [0, 64), so each row-reduction
becomes a GEMM against a label-derived matrix Q[l, g] followed by a per-row
one-hot select at g = targets[j].

Layout (g-major): per core (512-row j-shard, rotated so its own rows sit at
columns 0:512 of the l axis):

  for each of 32 l-tiles:
    sim   = xT_tile.T @ xT[:, 0:512]            # [128 l, 512 j]  PSUM
    y     = Ln(KSC * sim)                       # shifted log, bf16
    y2    = y * y                               # DVE bf16 2x
    X1a  += Qa.T @ y      (Qa = [P | W0])       # [128 g, 512 j]  accum
    X2a  += Qa.T @ y2                           # [128 g, 512 j]  accum
    X1b  += Qb.T @ y      (Qb = [W1])           # [ 64 g, 512 j]  accum

Q is STATIONARY and the full j-shard is the MOVING operand (N=512), so the
PE runs ~134 big matmuls instead of 320 small ones, and LDWEIGHTS drops
3x. All matmul operands are bf16 (the shift trick keeps y in [-0.3, 0.35],
so bf16's 8-bit mantissa costs only ~1e-4 absolute per element).

Selection: the 5 per-j values (yP, yW0, yW1, y2P, y2W0) are extracted
without transposes: multiply the [g, j] accumulators elementwise by the
one-hot mask M[g, j] = (g % 64 == t_j), then contract over partitions with a
tiny 2-column indicator matmul -> [2, 512] rows. Those 6 rows DMA out and
the final scalar loss is reconstructed on host in float64.
"""

import numpy as np
import ml_dtypes

N, D, KK, C = 4096, 128, 4, 64
NCORES = 8
JSH = N // NCORES          # 512 j rows per core
LT = N // 128              # 32 l-tiles
GW = 192                   # Q width: [P(64) | W0(64) | W1(64)]
EPS = 1e-6
OMEGA = 0.1
KSC = float(np.float32(np.exp(-3.5)))        # Ln input scale (exactly f32)
SHIFT = float(-np.log(np.float64(KSC)))      # effective shift s = -ln(KSC)

_CACHE = {}


def _build_nc():
    import concourse.bass as bass
    import concourse.bacc as bacc
    import concourse.mybir as mybir
    import concourse.tile as tile
    from contextlib import ExitStack

    f32 = mybir.dt.float32
    f32r = mybir.dt.float32r
    bf16 = mybir.dt.bfloat16
    Ln = mybir.ActivationFunctionType.Ln

    nc = bacc.Bacc("TRN2", target_bir_lowering=False, debug=False)
    xt = nc.dram_tensor("xt", [D, N], bf16, kind="ExternalInput")
    # q[p, lt*GW + g] = Q[lt*128 + p, g]
    q = nc.dram_tensor("q", [128, LT * GW], bf16, kind="ExternalInput")
    # mask[g, j] = (g % 64 == t_j), stacked twice along g
    msk = nc.dram_tensor("msk", [128, JSH], bf16, kind="ExternalInput")
    # indicator: ind[g, 0] = (g < 64), ind[g, 1] = (g >= 64)
    ind = nc.dram_tensor("ind", [128, 2], bf16, kind="ExternalInput")
    # out rows: [yP, yW0 | yW1, 0 | y2P, y2W0]
    lout = nc.dram_tensor("lout", [2, 3 * JSH], f32, kind="ExternalOutput")

    with tile.TileContext(nc) as tc, ExitStack() as ctx:
        cpool = ctx.enter_context(tc.tile_pool(name="const", bufs=1))
        work = ctx.enter_context(tc.tile_pool(name="work", bufs=4))
        mpool = ctx.enter_context(tc.tile_pool(name="mpool", bufs=1))
        psim = ctx.enter_context(tc.tile_pool(name="psim", bufs=3, space="PSUM"))
        px = ctx.enter_context(tc.tile_pool(name="px", bufs=1, space="PSUM"))
        psel = ctx.enter_context(tc.tile_pool(name="psel", bufs=1, space="PSUM"))

        # ---- PE warm-up: a memset scratch tile (no DMA dependency) feeds
        # back-to-back dummy matmuls from ~1.5us, so the HAM clock-gate opens
        # (1.2 -> 2.4 GHz) before real data lands and the real stream starts
        # warm. Output bank is never read.
        scr = cpool.tile([128, JSH], bf16, tag="scr")
        nc.vector.memset(scr[:], 0.0)
        warm = psel.tile([128, JSH], f32, tag="warm", name="warm")
        for _ in range(36):
            nc.tensor.matmul(
                warm[:], scr[:, 0:128], scr[:], start=True, stop=True
            )

        # ---- DMAs: first chunks feed the first sim matmuls; issue from two
        # engine queues (each DMA_DIRECT2D costs ~0.6us serial on its queue).
        xt_sb = cpool.tile([D, N], bf16, tag="xt")
        nc.sync.dma_start(xt_sb[:, 0:512], xt[:, 0:512])
        q_sb = cpool.tile([128, LT * GW], bf16, tag="q")
        nc.gpsimd.dma_start(q_sb[:, 0:1536], q[:, 0:1536])
        nc.sync.dma_start(xt_sb[:, 512:2304], xt[:, 512:2304])
        nc.gpsimd.dma_start(q_sb[:, 1536:3840], q[:, 1536:3840])
        nc.sync.dma_start(xt_sb[:, 2304:4096], xt[:, 2304:4096])
        nc.gpsimd.dma_start(q_sb[:, 3840:6144], q[:, 3840:6144])
        msk_sb = cpool.tile([128, JSH], bf16, tag="msk")
        nc.sync.dma_start(msk_sb[:], msk[:])
        ind_sb = cpool.tile([128, 2], bf16, tag="ind")
        nc.gpsimd.dma_start(ind_sb[:], ind[:])

        # accumulators (one PSUM bank each, held across the whole lt loop)
        x1a = px.tile([128, JSH], f32, tag="x1a")
        x2a = px.tile([128, JSH], f32, tag="x2a")
        x1b = px.tile([64, JSH], f32, tag="x1b")

        mov = xt_sb[:, 0:JSH]
        ls_t = [None] * LT
        ls2_t = [None] * LT

        def sim_stage(lt):
            simp = psim.tile([128, JSH], f32, tag="simp", name=f"simp{lt}")
            nc.tensor.matmul(
                simp[:], xt_sb[:, bass.ts(lt, 128)], mov, start=True, stop=True
            )
            ls = work.tile([128, JSH], bf16, tag="ls", name=f"ls{lt}")
            nc.scalar.activation(ls[:], simp[:], Ln, scale=KSC)
            ls2 = work.tile([128, JSH], bf16, tag="ls2", name=f"ls2{lt}")
            nc.vector.tensor_mul(ls2[:], ls[:], ls[:])
            ls_t[lt] = ls
            ls2_t[lt] = ls2

        for lt in range(3):
            sim_stage(lt)
        for lt in range(LT):
            if lt + 3 < LT:
                sim_stage(lt + 3)
            qa = q_sb[:, lt * GW : lt * GW + 128]
            qb = q_sb[:, lt * GW + 128 : lt * GW + GW]
            st = lt == 0
            sp = lt == LT - 1
            nc.tensor.matmul(x1a[:], qa, ls_t[lt][:], start=st, stop=sp)
            nc.tensor.matmul(x2a[:], qa, ls2_t[lt][:], start=st, stop=sp)
            nc.tensor.matmul(x1b[:], qb, ls_t[lt][:], start=st, stop=sp)

        # ---- selection: mask-mul then 2-column collapse matmul. The sel
        # PSUM tiles reuse the (now dead) accumulator banks via pool tags.
        sel_sb = mpool.tile([2, 3 * JSH], f32, tag="selsb")
        srcs = ((x1a, 128), (x1b, 64), (x2a, 128))
        ms = []
        for i, (src, prange) in enumerate(srcs):
            m = mpool.tile([prange, JSH], bf16, tag=f"m{i}", name=f"m{i}")
            nc.vector.tensor_mul(m[:], src[0:prange, :], msk_sb[0:prange, :])
            ms.append(m)
        for i, (src, prange) in enumerate(srcs):
            sel = px.tile([2, JSH], f32, tag=("x1a", "x1b", "x2a")[i],
                          name=f"sel{i}")
            nc.tensor.matmul(
                sel[:], ind_sb[0:prange, :], ms[i][:], start=True, stop=True
            )
            nc.vector.tensor_copy(sel_sb[:, bass.ts(i, JSH)], sel[:])
        nc.sync.dma_start(lout[:], sel_sb[:])
    nc.compile()
    return nc


def _host_prep(inputs, labels):
    x = np.asarray(inputs, dtype=np.float32)
    lab = np.asarray(labels)
    t = lab[:, 0].astype(np.int64)
    bf = ml_dtypes.bfloat16

    m = np.arange(KK)
    om = np.float64(OMEGA)
    lp = np.log(np.float64(OMEGA + EPS)) - np.log(om ** (KK - m + 1) + np.float64(EPS))

    gr = np.arange(C)
    eq = lab[None, :, :] == gr[:, None, None]          # [C, N, KK]
    nm = np.stack(
        [
            ~eq[:, :, 3],
            eq[:, :, 3] & ~eq[:, :, 2],
            eq[:, :, 2] & ~eq[:, :, 1],
            eq[:, :, 1] & ~eq[:, :, 0],
        ]
    ).astype(np.float64)                                # [KK, C, N]
    w0 = nm.sum(0)                                      # [C, N]
    w1 = np.einsum("m,mcl->cl", lp, nm)
    w2 = np.einsum("m,mcl->cl", lp * lp, nm)
    ph = (t[:, None] == gr[None, :]).astype(np.float64)  # [N, C] one-hot t_l

    qm = np.zeros((N, GW), dtype=np.float32)
    qm[:, 0:C] = ph
    qm[:, C : 2 * C] = w0.T
    qm[:, 2 * C : 3 * C] = w1.T

    ind = np.zeros((128, 2), dtype=np.float32)
    ind[0:64, 0] = 1.0
    ind[64:128, 1] = 1.0

    xt = np.ascontiguousarray(x.T)                       # [D, N]
    in_maps = []
    for cid in range(NCORES):
        sl = slice(cid * JSH, (cid + 1) * JSH)
        # rotate the l axis so this core's own j-shard sits at columns
        # 0:JSH; the l reduction (over all 4096) is rotation-invariant as
        # long as q's rows rotate identically.
        xtc = np.roll(xt, -cid * JSH, axis=1)
        qc = np.roll(qm, -cid * JSH, axis=0)             # [N, GW]
        # q_sb[p, lt*GW + g] = Q[lt*128 + p, g]
        qsb = np.ascontiguousarray(
            qc.reshape(LT, 128, GW).transpose(1, 0, 2).reshape(128, LT * GW)
        )
        oh = (gr[:, None] == t[sl][None, :]).astype(np.float32)  # [64, 512]
        mk = np.concatenate([oh, oh], axis=0)            # [128, 512]
        in_maps.append(
            {
                "xt": xtc.astype(bf),
                "q": qsb.astype(bf),
                "msk": mk.astype(bf),
                "ind": ind.astype(bf),
            }
        )

    tabs = {
        "t": t, "cnt": ph.sum(0), "h0": w0.sum(1), "h1": w1.sum(1),
        "h2": w2.sum(1), "x": x,
    }
    return in_maps, tabs


def _host_loss(res_list, tabs):
    t, cnt, h0, h1, h2 = tabs["t"], tabs["cnt"], tabs["h0"], tabs["h1"], tabs["h2"]
    x64 = tabs["x"].astype(np.float64)
    s = np.float64(SHIFT)
    loss = np.float64(0.0)
    for cid, r in enumerate(res_list):
        sl = slice(cid * JSH, (cid + 1) * JSH)
        lo = r["lout"].astype(np.float64)                # [2, 3*JSH]
        yP, yW0 = lo[0, 0:JSH], lo[1, 0:JSH]
        yW1 = lo[0, JSH : 2 * JSH]
        y2P, y2W0 = lo[0, 2 * JSH :], lo[1, 2 * JSH :]
        tj = t[sl]
        cj, h0j, h1j, h2j = cnt[tj], h0[tj], h1[tj], h2[tj]
        diag = np.log(np.einsum("jd,jd->j", x64[sl], x64[sl]) + EPS)
        S1 = yP + s * cj - diag
        S2 = y2P + 2 * s * yP + s * s * cj - diag * diag
        A1 = yW0 + s * h0j + 0.1 * h1j
        A2 = (y2W0 + 2 * s * yW0 + s * s * h0j) + 0.2 * (yW1 + s * h1j) + 0.01 * h2j
        loss += np.sum(S2 * h0j - 2.0 * S1 * A1 + (cj - 1.0) * A2)
    return np.array(loss, dtype=np.float32)


def _run(inputs, labels, trace=False, tmpdir=None):
    from concourse.bass_utils import run_bass_kernel_spmd

    if "nc" not in _CACHE:
        _CACHE["nc"] = _build_nc()
    in_maps, tabs = _host_prep(inputs, labels)
    res = run_bass_kernel_spmd(
        _CACHE["nc"], in_maps, core_ids=list(range(NCORES)),
        trace=trace, tmpdir=tmpdir,
    )
    return _host_loss(res.results, tabs), res


def kernel(inputs, labels):
    out, _ = _run(inputs, labels, trace=False)
    return out


# revision 9
# speedup vs baseline: 1.1271x; 1.0978x over previous
"""Trainium2 Bass kernel for the LogRatio loss (nn_LogRatio_14104672600201).

Math: the reference loss factorizes (see the derivation in the epilogue
below). Every masked reduction over logsim[j, l] = log((X @ X.T)[j, l]) has a
mask depending on j only through targets[j] in [0, 64), so each row-reduction
becomes a GEMM against a label-derived matrix Q[l, g] followed by a per-row
one-hot select at g = targets[j].

Layout (g-major): per core (512-row j-shard, rotated so its own rows sit at
columns 0:512 of the l axis):

  for each of 32 l-tiles:
    sim   = xT_tile.T @ xT[:, 0:512]            # [128 l, 512 j]  PSUM
    y     = Ln(KSC * sim)                       # shifted log, bf16
    y2    = y * y                               # DVE bf16 2x
    X1a  += Qa.T @ y      (Qa = [P | W0])       # [128 g, 512 j]  accum
    X2a  += Qa.T @ y2                           # [128 g, 512 j]  accum
    X1b  += Qb.T @ y      (Qb = [W1])           # [ 64 g, 512 j]  accum

Q is STATIONARY and the full j-shard is the MOVING operand (N=512), so the
PE runs ~134 big matmuls instead of 320 small ones, and LDWEIGHTS drops
3x. All matmul operands are bf16 (the shift trick keeps y in [-0.3, 0.35],
so bf16's 8-bit mantissa costs only ~1e-4 absolute per element).

Selection: the 5 per-j values (yP, yW0, yW1, y2P, y2W0) are extracted
without transposes: multiply the [g, j] accumulators elementwise by the
one-hot mask M[g, j] = (g % 64 == t_j), then contract over partitions with a
tiny 2-column indicator matmul -> [2, 512] rows. Those 6 rows DMA out and
the final scalar loss is reconstructed on host in float64.
"""

import numpy as np
import ml_dtypes

N, D, KK, C = 4096, 128, 4, 64
NCORES = 8
JSH = N // NCORES          # 512 j rows per core
LT = N // 128              # 32 l-tiles
GW = 192                   # Q width: [P(64) | W0(64) | W1(64)]
EPS = 1e-6
OMEGA = 0.1
KSC = float(np.float32(np.exp(-3.5)))        # Ln input scale (exactly f32)
SHIFT = float(-np.log(np.float64(KSC)))      # effective shift s = -ln(KSC)

_CACHE = {}


def _build_nc():
    import concourse.bass as bass
    import concourse.bacc as bacc
    import concourse.mybir as mybir
    import concourse.tile as tile
    from contextlib import ExitStack

    f32 = mybir.dt.float32
    f32r = mybir.dt.float32r
    bf16 = mybir.dt.bfloat16
    Ln = mybir.ActivationFunctionType.Ln

    nc = bacc.Bacc("TRN2", target_bir_lowering=False, debug=False)
    xt = nc.dram_tensor("xt", [D, N], bf16, kind="ExternalInput")
    # q[p, lt*GW + g] = Q[lt*128 + p, g]
    q = nc.dram_tensor("q", [128, LT * GW], bf16, kind="ExternalInput")
    # mask[g, j] = (g % 64 == t_j), stacked twice along g
    msk = nc.dram_tensor("msk", [128, JSH], bf16, kind="ExternalInput")
    # indicator: ind[g, 0] = (g < 64), ind[g, 1] = (g >= 64)
    ind = nc.dram_tensor("ind", [128, 2], bf16, kind="ExternalInput")
    # out rows: [yP, yW0 | yW1, 0 | y2P, y2W0]
    lout = nc.dram_tensor("lout", [2, 3 * JSH], f32, kind="ExternalOutput")

    with tile.TileContext(nc) as tc, ExitStack() as ctx:
        cpool = ctx.enter_context(tc.tile_pool(name="const", bufs=1))
        work = ctx.enter_context(tc.tile_pool(name="work", bufs=8))
        mpool = ctx.enter_context(tc.tile_pool(name="mpool", bufs=1))
        psim = ctx.enter_context(tc.tile_pool(name="psim", bufs=3, space="PSUM"))
        px = ctx.enter_context(tc.tile_pool(name="px", bufs=1, space="PSUM"))
        psel = ctx.enter_context(tc.tile_pool(name="psel", bufs=1, space="PSUM"))

        # ---- PE warm-up: a memset scratch tile (no DMA dependency) feeds
        # back-to-back dummy matmuls from ~1.5us, so the HAM clock-gate opens
        # (1.2 -> 2.4 GHz) before real data lands and the real stream starts
        # warm. Output bank is never read.
        scr = cpool.tile([128, JSH], bf16, tag="scr")
        nc.vector.memset(scr[:], 0.0)
        warm = psel.tile([128, JSH], f32, tag="warm", name="warm")
        for _ in range(4):
            nc.tensor.matmul(
                warm[:], scr[:, 0:128], scr[:], start=True, stop=True
            )

        # ---- DMAs: first chunks feed the first sim matmuls; issue from two
        # engine queues (each DMA_DIRECT2D costs ~0.6us serial on its queue).
        xt_sb = cpool.tile([D, N], bf16, tag="xt")
        nc.sync.dma_start(xt_sb[:, 0:512], xt[:, 0:512])
        q_sb = cpool.tile([128, LT * GW], bf16, tag="q")
        nc.gpsimd.dma_start(q_sb[:, 0:1536], q[:, 0:1536])
        nc.sync.dma_start(xt_sb[:, 512:2304], xt[:, 512:2304])
        nc.gpsimd.dma_start(q_sb[:, 1536:3840], q[:, 1536:3840])
        nc.sync.dma_start(xt_sb[:, 2304:4096], xt[:, 2304:4096])
        nc.gpsimd.dma_start(q_sb[:, 3840:6144], q[:, 3840:6144])
        msk_sb = cpool.tile([128, JSH], bf16, tag="msk")
        nc.sync.dma_start(msk_sb[:], msk[:])
        ind_sb = cpool.tile([128, 2], bf16, tag="ind")
        nc.gpsimd.dma_start(ind_sb[:], ind[:])

        # accumulators (one PSUM bank each, held across the whole lt loop)
        x1a = px.tile([128, JSH], f32, tag="x1a")
        x2a = px.tile([128, JSH], f32, tag="x2a")
        x1b = px.tile([64, JSH], f32, tag="x1b")

        mov = xt_sb[:, 0:JSH]
        ls_t = [None] * LT
        ls2_t = [None] * LT

        def sim_stage(lt):
            simp = psim.tile([128, JSH], f32, tag="simp", name=f"simp{lt}")
            nc.tensor.matmul(
                simp[:], xt_sb[:, bass.ts(lt, 128)], mov, start=True, stop=True
            )
            ls = work.tile([128, JSH], bf16, tag="ls", name=f"ls{lt}")
            nc.scalar.activation(ls[:], simp[:], Ln, scale=KSC)
            ls2 = work.tile([128, JSH], bf16, tag="ls2", name=f"ls2{lt}")
            nc.vector.tensor_mul(ls2[:], ls[:], ls[:])
            ls_t[lt] = ls
            ls2_t[lt] = ls2

        for lt in range(3):
            sim_stage(lt)
        for lt in range(LT):
            if lt + 3 < LT:
                sim_stage(lt + 3)
            qa = q_sb[:, lt * GW : lt * GW + 128]
            qb = q_sb[:, lt * GW + 128 : lt * GW + GW]
            st = lt == 0
            sp = lt == LT - 1
            nc.tensor.matmul(x1a[:], qa, ls_t[lt][:], start=st, stop=sp)
            nc.tensor.matmul(x2a[:], qa, ls2_t[lt][:], start=st, stop=sp)
            nc.tensor.matmul(x1b[:], qb, ls_t[lt][:], start=st, stop=sp)

        # ---- selection: mask-mul then 2-column collapse matmul. The sel
        # PSUM tiles reuse the (now dead) accumulator banks via pool tags.
        sel_sb = mpool.tile([2, 3 * JSH], f32, tag="selsb")
        srcs = ((x1a, 128), (x1b, 64), (x2a, 128))
        ms = []
        for i, (src, prange) in enumerate(srcs):
            m = mpool.tile([prange, JSH], bf16, tag=f"m{i}", name=f"m{i}")
            nc.vector.tensor_mul(m[:], src[0:prange, :], msk_sb[0:prange, :])
            ms.append(m)
        for i, (src, prange) in enumerate(srcs):
            sel = px.tile([2, JSH], f32, tag=("x1a", "x1b", "x2a")[i],
                          name=f"sel{i}")
            nc.tensor.matmul(
                sel[:], ind_sb[0:prange, :], ms[i][:], start=True, stop=True
            )
            nc.vector.tensor_copy(sel_sb[:, bass.ts(i, JSH)], sel[:])
        nc.sync.dma_start(lout[:], sel_sb[:])
    nc.compile()
    return nc


def _host_prep(inputs, labels):
    x = np.asarray(inputs, dtype=np.float32)
    lab = np.asarray(labels)
    t = lab[:, 0].astype(np.int64)
    bf = ml_dtypes.bfloat16

    m = np.arange(KK)
    om = np.float64(OMEGA)
    lp = np.log(np.float64(OMEGA + EPS)) - np.log(om ** (KK - m + 1) + np.float64(EPS))

    gr = np.arange(C)
    eq = lab[None, :, :] == gr[:, None, None]          # [C, N, KK]
    nm = np.stack(
        [
            ~eq[:, :, 3],
            eq[:, :, 3] & ~eq[:, :, 2],
            eq[:, :, 2] & ~eq[:, :, 1],
            eq[:, :, 1] & ~eq[:, :, 0],
        ]
    ).astype(np.float64)                                # [KK, C, N]
    w0 = nm.sum(0)                                      # [C, N]
    w1 = np.einsum("m,mcl->cl", lp, nm)
    w2 = np.einsum("m,mcl->cl", lp * lp, nm)
    ph = (t[:, None] == gr[None, :]).astype(np.float64)  # [N, C] one-hot t_l

    qm = np.zeros((N, GW), dtype=np.float32)
    qm[:, 0:C] = ph
    qm[:, C : 2 * C] = w0.T
    qm[:, 2 * C : 3 * C] = w1.T

    ind = np.zeros((128, 2), dtype=np.float32)
    ind[0:64, 0] = 1.0
    ind[64:128, 1] = 1.0

    xt = np.ascontiguousarray(x.T)                       # [D, N]
    in_maps = []
    for cid in range(NCORES):
        sl = slice(cid * JSH, (cid + 1) * JSH)
        # rotate the l axis so this core's own j-shard sits at columns
        # 0:JSH; the l reduction (over all 4096) is rotation-invariant as
        # long as q's rows rotate identically.
        xtc = np.roll(xt, -cid * JSH, axis=1)
        qc = np.roll(qm, -cid * JSH, axis=0)             # [N, GW]
        # q_sb[p, lt*GW + g] = Q[lt*128 + p, g]
        qsb = np.ascontiguousarray(
            qc.reshape(LT, 128, GW).transpose(1, 0, 2).reshape(128, LT * GW)
        )
        oh = (gr[:, None] == t[sl][None, :]).astype(np.float32)  # [64, 512]
        mk = np.concatenate([oh, oh], axis=0)            # [128, 512]
        in_maps.append(
            {
                "xt": xtc.astype(bf),
                "q": qsb.astype(bf),
                "msk": mk.astype(bf),
                "ind": ind.astype(bf),
            }
        )

    tabs = {
        "t": t, "cnt": ph.sum(0), "h0": w0.sum(1), "h1": w1.sum(1),
        "h2": w2.sum(1), "x": x,
    }
    return in_maps, tabs


def _host_loss(res_list, tabs):
    t, cnt, h0, h1, h2 = tabs["t"], tabs["cnt"], tabs["h0"], tabs["h1"], tabs["h2"]
    x64 = tabs["x"].astype(np.float64)
    s = np.float64(SHIFT)
    loss = np.float64(0.0)
    for cid, r in enumerate(res_list):
        sl = slice(cid * JSH, (cid + 1) * JSH)
        lo = r["lout"].astype(np.float64)                # [2, 3*JSH]
        yP, yW0 = lo[0, 0:JSH], lo[1, 0:JSH]
        yW1 = lo[0, JSH : 2 * JSH]
        y2P, y2W0 = lo[0, 2 * JSH :], lo[1, 2 * JSH :]
        tj = t[sl]
        cj, h0j, h1j, h2j = cnt[tj], h0[tj], h1[tj], h2[tj]
        diag = np.log(np.einsum("jd,jd->j", x64[sl], x64[sl]) + EPS)
        S1 = yP + s * cj - diag
        S2 = y2P + 2 * s * yP + s * s * cj - diag * diag
        A1 = yW0 + s * h0j + 0.1 * h1j
        A2 = (y2W0 + 2 * s * yW0 + s * s * h0j) + 0.2 * (yW1 + s * h1j) + 0.01 * h2j
        loss += np.sum(S2 * h0j - 2.0 * S1 * A1 + (cj - 1.0) * A2)
    return np.array(loss, dtype=np.float32)


def _run(inputs, labels, trace=False, tmpdir=None):
    from concourse.bass_utils import run_bass_kernel_spmd

    if "nc" not in _CACHE:
        _CACHE["nc"] = _build_nc()
    in_maps, tabs = _host_prep(inputs, labels)
    res = run_bass_kernel_spmd(
        _CACHE["nc"], in_maps, core_ids=list(range(NCORES)),
        trace=trace, tmpdir=tmpdir,
    )
    return _host_loss(res.results, tabs), res


def kernel(inputs, labels):
    out, _ = _run(inputs, labels, trace=False)
    return out
